# revision 16
# baseline (speedup 1.0000x reference)
"""Trainium2 Bass kernel for nn_EventPairCompositionModel.

Strategy (data-parallel over batch, 8 cores, B=512 -> 64 per core):
  - Host compacts the 60MB f32 table per core to the ~24K unique rows its
    shard touches (bf16, rows padded to 384 elems = 768B), remapping all
    indices to int16.  The device then uses the fast SWDGE dma_gather
    (InstDMAGatherAnt) to fetch context/event embeddings.
  - XBAR DMA transpose (SBUF->SBUF) turns gathered bn-major rows into
    K-major tiles for the tensor engine.
  - Shared arg-composition MLP (1536->512->256, zero-padded K) in bf16.
  - Cosine numerators/denominators via small per-b matmuls that land
    n-on-partitions; norms folded together through one exp(-0.5 ln x).
  - KNRM kernel pooling via ones-matmul partition reductions, distance
    kernel path, final linear + sigmoid, all on-chip.
  - If a shard ever touches >32767 unique rows (can't happen for random
    inputs), falls back to a slow indirect-DMA gather of the full table.
All 8 cores run the identical program on their own batch shard (SPMD, no
collectives); host concatenates the 8 (64,1) outputs.
"""

import numpy as np
import ml_dtypes

import concourse.bacc as bacc
import concourse.bass as bass
import concourse.tile as tile
import concourse.mybir as mybir
from concourse.bass import IndirectOffsetOnAxis
from concourse.bass_utils import run_bass_kernel_spmd
from concourse import library_config

F32 = mybir.dt.float32
BF16 = mybir.dt.bfloat16
I16 = mybir.dt.int16
I32 = mybir.dt.int32
AF = mybir.ActivationFunctionType

# Problem shapes (hardcoded per spec)
B, N, C, E = 512, 128, 4, 300
V = 50000
H1, H2 = 512, 256
NF, NK = 8, 11
NCORES = 8
BC = B // NCORES          # 64 batches per core
EP = 384                  # padded embedding stride inside an x-row (768B)
CE = C * EP               # 1536 padded x-row length
KT = CE // 128            # 12 K-tiles for MLP1
CT = 32768                # compact table rows (int16-indexable)
GROUPS = (BC * N) // 512  # 16 groups of 512 (b,n) pairs
SUBT = 4                  # 128-bn subtiles per group
EB = 128                  # event-path width (64 real b + 64 junk)

MUS = [1.0, 0.9, 0.7, 0.5, 0.3, 0.1, -0.1, -0.3, -0.5, -0.7, -0.9]
SIGMAS = [1e-3] + [0.1] * 10

_PROGRAM_CACHE = {}


def _build_program(fast: bool):
    if fast in _PROGRAM_CACHE:
        return _PROGRAM_CACHE[fast]

    nc = bacc.Bacc("TRN2", target_bir_lowering=False, debug=False, num_swdge_queues=4)

    # ---- DRAM I/O ----
    if fast:
        ctab = nc.dram_tensor("ctab", (CT, EP), BF16, kind="ExternalInput")
        cidx = nc.dram_tensor("cidx", (128, GROUPS * 128), I16, kind="ExternalInput")
        eidx = nc.dram_tensor("eidx", (128, 32), I16, kind="ExternalInput")
    else:
        ctab = nc.dram_tensor("table", (V + 1, E), F32, kind="ExternalInput")
        cidx = nc.dram_tensor("ctxidx", (128, BC * C), I32, kind="ExternalInput")
        eidx = nc.dram_tensor("evidx", (BC, C), I32, kind="ExternalInput")
    w1t = nc.dram_tensor("w1t", (CE, H1), BF16, kind="ExternalInput")
    w2t = nc.dram_tensor("w2t", (H1, H2), BF16, kind="ExternalInput")
    wvt = nc.dram_tensor("wvt", (CE, 9), BF16, kind="ExternalInput")
    b1d = nc.dram_tensor("b1d", (128, 4), F32, kind="ExternalInput")
    b2d = nc.dram_tensor("b2d", (128, 2), F32, kind="ExternalInput")
    bvd = nc.dram_tensor("bvd", (9, 1), F32, kind="ExternalInput")
    wct = nc.dram_tensor("wct", (128, 1), F32, kind="ExternalInput")
    wckp = nc.dram_tensor("wckp", (1, NK), F32, kind="ExternalInput")
    bcd = nc.dram_tensor("bcd", (1, 1), F32, kind="ExternalInput")
    ndsq = nc.dram_tensor("ndsq", (9, BC), F32, kind="ExternalInput")
    featT = nc.dram_tensor("featT", (NF, BC), F32, kind="ExternalInput")
    out_d = nc.dram_tensor("out", (BC, 1), F32, kind="ExternalOutput")

    with tile.TileContext(nc) as tc:
        with (
            tc.tile_pool(name="consts", bufs=1) as cpool,
            tc.tile_pool(name="xg", bufs=4) as xgpool,
            tc.tile_pool(name="xt", bufs=4) as xtpool,
            tc.tile_pool(name="s1", bufs=8) as s1pool,
            tc.tile_pool(name="s2", bufs=4) as s2pool,
            tc.tile_pool(name="csq", bufs=4) as csqpool,
            tc.tile_pool(name="small", bufs=2) as smpool,
            tc.tile_pool(name="pm1", bufs=2, space="PSUM") as pm1,
            tc.tile_pool(name="pm2", bufs=2, space="PSUM") as pm2,
            tc.tile_pool(name="ptn", bufs=1, space="PSUM") as ptn,
            tc.tile_pool(name="pmisc", bufs=2, space="PSUM") as pmisc,
        ):
            # ---- load constants ----
            if fast:
                nc.gpsimd.load_library(library_config.mlp)
            w1t_s = cpool.tile([128, KT * H1], BF16)
            nc.sync.dma_start(
                w1t_s[:].rearrange("p (t m) -> p t m", t=KT),
                w1t.ap().rearrange("(t p) m -> p t m", p=128),
            )
            w2t_s = cpool.tile([128, 4 * H2], BF16)
            nc.sync.dma_start(
                w2t_s[:].rearrange("p (t m) -> p t m", t=4),
                w2t.ap().rearrange("(t p) m -> p t m", p=128),
            )
            wvt_s = cpool.tile([128, KT * 9], BF16)
            nc.sync.dma_start(
                wvt_s[:].rearrange("p (t m) -> p t m", t=KT),
                wvt.ap().rearrange("(t p) m -> p t m", p=128),
            )
            b1_s = cpool.tile([128, 4], F32)
            nc.sync.dma_start(b1_s[:], b1d.ap())
            b2_s = cpool.tile([128, 2], F32)
            nc.sync.dma_start(b2_s[:], b2d.ap())
            bv_s = cpool.tile([9, 1], F32)
            nc.sync.dma_start(bv_s[:], bvd.ap())
            wct_s = cpool.tile([128, 1], F32)
            nc.sync.dma_start(wct_s[:], wct.ap())
            wckp_s = cpool.tile([1, NK], F32)
            nc.sync.dma_start(wckp_s[:], wckp.ap())
            bc_s = cpool.tile([1, 1], F32)
            nc.sync.dma_start(bc_s[:], bcd.ap())
            if fast:
                cidx_s = cpool.tile([128, GROUPS * 128], I16)
                nc.sync.dma_start(cidx_s[:], cidx.ap())
                eidx_s = cpool.tile([128, 32], I16)
                nc.sync.dma_start(eidx_s[:], eidx.ap())
            else:
                cidx_s = cpool.tile([128, BC * C], I32)
                nc.sync.dma_start(cidx_s[:], cidx.ap())
                eidx_s = cpool.tile([BC, C], I32)
                nc.sync.dma_start(eidx_s[:], eidx.ap())
            ndsq_s = cpool.tile([9, BC], F32)
            nc.sync.dma_start(ndsq_s[:], ndsq.ap())
            feat_s = cpool.tile([128, BC], F32)
            nc.vector.memset(feat_s[:], 0.0)
            nc.sync.dma_start(feat_s[64 : 64 + NF, :], featT.ap())
            ones_s = cpool.tile([128, 1], BF16)
            nc.vector.memset(ones_s[:], 1.0)
            onesrow_s = cpool.tile([1, 128], F32)
            nc.vector.memset(onesrow_s[:], 1.0)
            onesf_s = cpool.tile([128, 1], F32)
            nc.vector.memset(onesf_s[:], 1.0)
            eps_s = cpool.tile([128, 1], F32)
            nc.vector.memset(eps_s[:], 1e-20)
            mub_s = cpool.tile([128, NK], F32)
            for k in range(NK):
                nc.vector.memset(mub_s[:, k : k + 1], -MUS[k])

            # ---- event path (EB=128 lanes, only 0..63 meaningful) ----
            xe = cpool.tile([EB, CE], BF16)
            if fast:
                nc.gpsimd.dma_gather(
                    out_ap=xe[:].rearrange("p (t e) -> p t e", t=C),
                    in_ap=ctab.ap(),
                    idxs_ap=eidx_s[:],
                    num_idxs=512,
                    num_idxs_reg=512,
                    elem_size=EP,
                    queue_num=3,
                )
            else:
                nc.vector.memset(xe[:], 0.0)
                nc.gpsimd.indirect_dma_start(
                    out=xe[0:BC, :].rearrange("p (c e) -> p c e", c=C)[:, :, 0:E],
                    out_offset=None,
                    in_=ctab.ap(),
                    in_offset=IndirectOffsetOnAxis(ap=eidx_s[:], axis=0),
                )
            xeT = cpool.tile([128, KT * EB], BF16)
            nc.sync.dma_start_transpose(
                xeT[:].rearrange("p (j i) -> p j i", j=KT), xe[:]
            )

            s1e = cpool.tile([128, 4 * EB], BF16)
            for m in range(4):
                pe = pmisc.tile([128, EB], F32, tag="pmisc", name="pe")
                for j in range(KT):
                    nc.tensor.matmul(
                        pe[:],
                        w1t_s[:, H1 * j + 128 * m : H1 * j + 128 * m + 128],
                        xeT[:, EB * j : EB * (j + 1)],
                        start=(j == 0),
                        stop=(j == KT - 1),
                    )
                nc.scalar.activation(
                    s1e[:, EB * m : EB * (m + 1)], pe[:], AF.Relu,
                    bias=b1_s[:, m : m + 1],
                )

            eh2 = [
                cpool.tile([128, EB], BF16, tag=f"eh2_{k}", name=f"eh2_{k}")
                for k in range(2)
            ]
            for m in range(2):
                pe2 = pmisc.tile([128, EB], F32, tag="pmisc", name="pe2")
                for j in range(4):
                    nc.tensor.matmul(
                        pe2[:],
                        w2t_s[:, H2 * j + 128 * m : H2 * j + 128 * m + 128],
                        s1e[:, EB * j : EB * (j + 1)],
                        start=(j == 0),
                        stop=(j == 3),
                    )
                nc.scalar.activation(
                    eh2[m][:], pe2[:], AF.Relu, bias=b2_s[:, m : m + 1]
                )

            # variances -> dist_emb rows 32..40 of feat_s
            pv = pmisc.tile([9, EB], F32, tag="pmisc", name="pv")
            for j in range(KT):
                nc.tensor.matmul(
                    pv[:],
                    wvt_s[:, 9 * j : 9 * (j + 1)],
                    xeT[:, EB * j : EB * (j + 1)],
                    start=(j == 0),
                    stop=(j == KT - 1),
                )
            ez_s = smpool.tile([9, EB], F32)
            nc.scalar.activation(ez_s[:], pv[:], AF.Exp, bias=bv_s[:])
            ez1_s = smpool.tile([9, EB], F32)
            nc.vector.tensor_scalar_add(ez1_s[:], ez_s[:], 1.0)
            var_s = smpool.tile([9, EB], F32)
            nc.scalar.activation(var_s[:], ez1_s[:], AF.Ln)
            rv_s = smpool.tile([9, EB], F32)
            nc.vector.reciprocal(rv_s[:], var_s[:])
            q_s = smpool.tile([9, BC], F32)
            nc.vector.tensor_mul(q_s[:], ndsq_s[:], rv_s[:, 0:BC])
            nc.scalar.activation(feat_s[32:41, :], q_s[:], AF.Exp)

            # |e|^2 per b, broadcast to all 128 partitions via outer product
            esq = [
                smpool.tile([128, EB], BF16, tag=f"esq_{k}", name=f"esq_{k}")
                for k in range(2)
            ]
            for k in range(2):
                nc.vector.tensor_mul(esq[k][:], eh2[k][:], eh2[k][:])
            pne = pmisc.tile([1, EB], F32, tag="pmisc", name="pne")
            for k in range(2):
                nc.tensor.matmul(
                    pne[:], ones_s[:], esq[k][:], start=(k == 0), stop=(k == 1)
                )
            ne2_s = smpool.tile([1, BC], F32)
            nc.scalar.copy(ne2_s[:], pne[:, 0:BC])
            pne2bc = pmisc.tile([128, BC], F32, tag="pmisc", name="pne2bc")
            nc.tensor.matmul(
                pne2bc[:], onesrow_s[:], ne2_s[:], start=True, stop=True
            )
            ne2bc_s = cpool.tile([128, BC], F32)
            nc.scalar.copy(ne2bc_s[:], pne2bc[:])

            # persistent SBUF accumulators, n on partitions, b on free
            traw_s = cpool.tile([128, BC], F32)
            ncsq_s = cpool.tile([128, BC], F32)

            # ---- context groups ----
            for g in range(GROUPS):
                xg = xgpool.tile([128, SUBT * CE], BF16)
                if fast:
                    for h in range(2):
                        nc.gpsimd.dma_gather(
                            out_ap=xg[:, 2 * CE * h : 2 * CE * (h + 1)]
                            .rearrange("p (t e) -> p t e", t=8),
                            in_ap=ctab.ap(),
                            idxs_ap=cidx_s[
                                :, 128 * g + 64 * h : 128 * g + 64 * (h + 1)
                            ],
                            num_idxs=1024,
                            num_idxs_reg=1024,
                            elem_size=EP,
                            queue_num=(2 * g + h) % 3,
                        )
                else:
                    nc.vector.memset(
                        xg[:].rearrange("p (q e) -> p q e", e=EP)[:, :, E:EP],
                        0.0,
                    )
                    for s in range(SUBT):
                        nc.gpsimd.indirect_dma_start(
                            out=xg[:]
                            .rearrange("p (q c e) -> p q c e", q=SUBT, c=C)[
                                :, s, :, 0:E
                            ],
                            out_offset=None,
                            in_=ctab.ap(),
                            in_offset=IndirectOffsetOnAxis(
                                ap=cidx_s[
                                    :, (SUBT * g + s) * C : (SUBT * g + s + 1) * C
                                ],
                                axis=0,
                            ),
                        )
                xt = xtpool.tile([128, KT * 512], BF16)
                for s in range(SUBT):
                    nc.sync.dma_start_transpose(
                        xt[:].rearrange("p (j z i) -> p j z i", j=KT, z=SUBT)[
                            :, :, s, :
                        ],
                        xg[:, CE * s : CE * (s + 1)],
                    )

                s1 = [
                    s1pool.tile([128, 512], BF16, tag=f"s1_{m}", name=f"s1_{m}")
                    for m in range(4)
                ]
                for m in range(4):
                    p1 = pm1.tile([128, 512], F32)
                    for j in range(KT):
                        nc.tensor.matmul(
                            p1[:],
                            w1t_s[:, H1 * j + 128 * m : H1 * j + 128 * m + 128],
                            xt[:, 512 * j : 512 * (j + 1)],
                            start=(j == 0),
                            stop=(j == KT - 1),
                        )
                    nc.scalar.activation(
                        s1[m][:], p1[:], AF.Relu, bias=b1_s[:, m : m + 1]
                    )

                s2 = [
                    s2pool.tile([128, 512], BF16, tag=f"s2_{m}", name=f"s2_{m}")
                    for m in range(2)
                ]
                for m in range(2):
                    p2 = pm2.tile([128, 512], F32)
                    for j in range(4):
                        nc.tensor.matmul(
                            p2[:],
                            w2t_s[:, H2 * j + 128 * m : H2 * j + 128 * m + 128],
                            s1[j][:],
                            start=(j == 0),
                            stop=(j == 3),
                        )
                    nc.scalar.activation(
                        s2[m][:], p2[:], AF.Relu, bias=b2_s[:, m : m + 1]
                    )

                csq = [
                    csqpool.tile([128, 512], BF16, tag=f"csq_{m}", name=f"csq_{m}")
                    for m in range(2)
                ]
                for m in range(2):
                    nc.vector.tensor_mul(csq[m][:], s2[m][:], s2[m][:])

                # raw dots and |c|^2, n on partitions, one column per b
                pT = ptn.tile([128, SUBT], F32, tag="pT", name="pT")
                pN = ptn.tile([128, SUBT], F32, tag="pN", name="pN")
                for s in range(SUBT):
                    b = SUBT * g + s
                    for k in range(2):
                        nc.tensor.matmul(
                            pT[:, s : s + 1],
                            s2[k][:, 128 * s : 128 * (s + 1)],
                            eh2[k][:, b : b + 1],
                            start=(k == 0),
                            stop=(k == 1),
                        )
                    for k in range(2):
                        nc.tensor.matmul(
                            pN[:, s : s + 1],
                            csq[k][:, 128 * s : 128 * (s + 1)],
                            ones_s[:],
                            start=(k == 0),
                            stop=(k == 1),
                        )
                nc.scalar.copy(traw_s[:, SUBT * g : SUBT * (g + 1)], pT[:])
                nc.scalar.copy(ncsq_s[:, SUBT * g : SUBT * (g + 1)], pN[:])

            # ---- kernel pooling (tiles are [n=128, b=64]) ----
            prodn = smpool.tile([128, BC], F32, tag="prodn")
            nc.vector.tensor_mul(prodn[:], ncsq_s[:], ne2bc_s[:])
            lnp = smpool.tile([128, BC], F32, tag="lnp")
            nc.scalar.activation(lnp[:], prodn[:], AF.Ln, bias=eps_s[:])
            nrmf = smpool.tile([128, BC], F32, tag="nrmf")
            nc.scalar.activation(nrmf[:], lnp[:], AF.Exp, scale=-0.5)
            trans = cpool.tile([128, BC], F32)
            nc.vector.tensor_mul(trans[:], traw_s[:], nrmf[:])

            kpp_s = cpool.tile([1, NK * BC], F32)
            for k in range(NK):
                sq = smpool.tile([128, BC], F32, tag="sq", name="sq")
                nc.scalar.activation(
                    sq[:], trans[:], AF.Square, bias=mub_s[:, k : k + 1]
                )
                arg = smpool.tile([128, BC], F32, tag="arg", name="arg")
                nc.vector.tensor_scalar(
                    arg[:], sq[:],
                    -1.0 / (2.0 * SIGMAS[k] ** 2), -87.0,
                    mybir.AluOpType.mult, mybir.AluOpType.max,
                )
                ek = smpool.tile([128, BC], F32, tag="ek", name="ek")
                nc.scalar.activation(ek[:], arg[:], AF.Exp)
                pp = pmisc.tile([1, BC], F32, tag="pmisc", name="pp")
                nc.tensor.matmul(pp[:], onesf_s[:], ek[:], start=True, stop=True)
                nc.scalar.copy(kpp_s[:, BC * k : BC * (k + 1)], pp[:])

            kpc_s = smpool.tile([1, NK * BC], F32, tag="kpc")
            nc.vector.tensor_scalar_max(kpc_s[:], kpp_s[:], 1e-10)
            kpl_s = smpool.tile([1, NK * BC], F32, tag="kpl")
            nc.scalar.activation(kpl_s[:], kpc_s[:], AF.Ln)

            # weighted sum over k: kps[b] = sum_k wckp[k] * kpl[k, b]
            kpw_s = smpool.tile([1, BC * NK], F32, tag="kpw")
            kpl_v = kpl_s[:].rearrange("p (k b) -> p b k", k=NK)
            wck_v = wckp_s[:][:, None, :].broadcast_to([1, BC, NK])
            kpw_v = kpw_s[:].rearrange("p (b k) -> p b k", b=BC)
            nc.vector.tensor_tensor(
                out=kpw_v, in0=kpl_v, in1=wck_v, op=mybir.AluOpType.mult
            )
            kps_s = smpool.tile([1, BC], F32, tag="kps")
            nc.vector.reduce_sum(
                out=kps_s[:], in_=kpw_v, axis=mybir.AxisListType.X
            )

            # ---- final score ----
            psc = pmisc.tile([1, BC], F32, tag="pmisc", name="psc")
            nc.tensor.matmul(psc[:], wct_s[:], feat_s[:], start=True, stop=True)
            tot_s = smpool.tile([1, BC], F32, tag="tot")
            nc.vector.tensor_add(tot_s[:], psc[:], kps_s[:])
            emx = smpool.tile([1, BC], F32, tag="emx")
            nc.scalar.activation(emx[:], tot_s[:], AF.Exp, bias=bc_s[:], scale=-1.0)
            emx1 = smpool.tile([1, BC], F32, tag="emx1")
            nc.vector.tensor_scalar_add(emx1[:], emx[:], 1.0)
            outs = smpool.tile([1, BC], F32, tag="outs")
            nc.vector.reciprocal(outs[:], emx1[:])
            nc.sync.dma_start(out_d.ap().rearrange("b one -> one b"), outs[:])

    nc.compile()
    _PROGRAM_CACHE[fast] = nc
    return nc


def _wrap16(flat_idx):
    """int16 index list -> (128, n/16) tile layout: unwrapped[i] =
    tile[i % 16, i // 16], replicated into all 8 16-partition stripes."""
    n = flat_idx.shape[0]
    t = np.zeros((16, n // 16), np.int16)
    t[np.arange(n) % 16, np.arange(n) // 16] = flat_idx
    return np.tile(t, (8, 1))


def _prep_core_inputs(inputs, core, fast):
    """Host-side shard + weight re-layouts for one core."""
    W1 = np.asarray(inputs["W1"], np.float32)
    W2 = np.asarray(inputs["W2"], np.float32)
    Wv = np.asarray(inputs["Wv"], np.float32)
    Wc = np.asarray(inputs["Wc"], np.float32)
    b1 = np.asarray(inputs["b1"], np.float32)
    b2 = np.asarray(inputs["b2"], np.float32)
    bv = np.asarray(inputs["bv"], np.float32)
    bc = np.asarray(inputs["bc"], np.float32)

    sl = slice(core * BC, (core + 1) * BC)
    ev = np.asarray(inputs["batch_event"][sl], np.int64)          # (BC, C)
    feats = np.asarray(inputs["batch_features"][sl], np.float32)  # (BC, NF)
    dists = np.asarray(inputs["batch_distances"][sl], np.float32) # (BC, 9)
    ctx = np.asarray(inputs["batch_context"][sl], np.int64)       # (BC, N, C)

    bf = ml_dtypes.bfloat16
    # W1.T with K padded 300->EP per component, zeros in the pad rows
    w1t = np.zeros((CE, H1), np.float32)
    for c in range(C):
        w1t[EP * c : EP * c + E, :] = W1[:, E * c : E * (c + 1)].T
    wvt = np.zeros((CE, 9), np.float32)
    wvt[EP * 1 : EP * 1 + E, :] = Wv.T  # predicates = component 1

    wc_full = np.zeros((128,), np.float32)
    wc_full[32 : 32 + 9] = Wc[0, 0:9]          # dist_emb block
    wc_full[64 : 64 + NF] = Wc[0, 9 : 9 + NF]  # batch_features block
    wckp = (Wc[0, NF + 9 :] * 0.01).astype(np.float32)  # kp block, 0.01 folded

    m = {
        "w1t": w1t.astype(bf),
        "w2t": np.ascontiguousarray(W2.T).astype(bf),
        "wvt": wvt.astype(bf),
        "b1d": np.ascontiguousarray(b1.reshape(4, 128).T),
        "b2d": np.ascontiguousarray(b2.reshape(2, 128).T),
        "bvd": bv.reshape(9, 1),
        "wct": wc_full.reshape(-1, 1),
        "wckp": wckp.reshape(1, NK),
        "bcd": -bc.reshape(1, 1),
        "ndsq": np.ascontiguousarray(-(dists * dists).T),
        "featT": np.ascontiguousarray(feats.T),
    }

    if fast:
        table = np.asarray(inputs["event_table"])
        allidx = np.concatenate([ctx.reshape(-1), ev.reshape(-1)])
        uniq, inv = np.unique(allidx, return_inverse=True)
        assert len(uniq) <= CT
        ctab = np.zeros((CT, EP), bf)
        ctab[: len(uniq), :E] = np.asarray(table[uniq], np.float32)
        rctx = inv[: ctx.size].astype(np.int16).reshape(BC, N, C)
        rev = inv[ctx.size :].astype(np.int16).reshape(BC, C)

        # context: gather i = (4s+c)*128 + p  <-  ctx[4g+s, p, c]
        # (BC,N,C) -> per group (SUBT,128,C) -> order (s,c,p)
        ci = rctx.reshape(GROUPS, SUBT, N, C).transpose(0, 1, 3, 2)  # g,s,c,p
        cidx = np.concatenate(
            [
                _wrap16(ci[g, 2 * h : 2 * h + 2].reshape(-1))
                for g in range(GROUPS)
                for h in range(2)
            ],
            axis=1,
        )
        # event: i = c*128 + b; b >= BC -> row 0 junk
        ei = np.zeros((C, 128), np.int16)
        ei[:, :BC] = rev.T
        m["ctab"] = ctab
        m["cidx"] = np.ascontiguousarray(cidx)
        m["eidx"] = np.ascontiguousarray(_wrap16(ei.reshape(-1)))
    else:
        m["table"] = np.ascontiguousarray(
            np.asarray(inputs["event_table"], np.float32)
        )
        m["ctxidx"] = np.ascontiguousarray(
            ctx.astype(np.int32).transpose(1, 0, 2).reshape(128, BC * C)
        )
        m["evidx"] = ev.astype(np.int32)
    return m


def kernel(**inputs) -> np.ndarray:
    # fast path requires every shard's unique row count to fit int16
    fast = True
    ctx = np.asarray(inputs["batch_context"], np.int64)
    ev = np.asarray(inputs["batch_event"], np.int64)
    for core in range(NCORES):
        sl = slice(core * BC, (core + 1) * BC)
        nuniq = len(np.unique(np.concatenate(
            [ctx[sl].reshape(-1), ev[sl].reshape(-1)])))
        if nuniq > CT:
            fast = False
            break
    nc = _build_program(fast)
    in_maps = [_prep_core_inputs(inputs, core, fast) for core in range(NCORES)]
    res = run_bass_kernel_spmd(nc, in_maps, core_ids=list(range(NCORES)))
    return np.concatenate([r["out"] for r in res.results], axis=0)


if __name__ == "__main__":
    nc = _build_program(True)
    print("program built ok")


# revision 19
# speedup vs baseline: 2.0562x; 2.0562x over previous
"""Trainium2 Bass kernel for nn_EventPairCompositionModel.

Strategy (data-parallel over batch, 8 cores, B=512 -> 64 per core):
  - Host compacts the 60MB f32 table per core to the ~24K unique rows its
    shard touches (bf16, rows padded to 384 elems = 768B), remapping all
    indices to int16.  The device then uses the fast SWDGE dma_gather
    (InstDMAGatherAnt) to fetch context/event embeddings.
  - XBAR DMA transpose (SBUF->SBUF) turns gathered bn-major rows into
    K-major tiles for the tensor engine.
  - Shared arg-composition MLP (1536->512->256, zero-padded K) in bf16.
  - Cosine numerators/denominators via small per-b matmuls that land
    n-on-partitions; norms folded together through one exp(-0.5 ln x).
  - KNRM kernel pooling via ones-matmul partition reductions, distance
    kernel path, final linear + sigmoid, all on-chip.
  - If a shard ever touches >32767 unique rows (can't happen for random
    inputs), falls back to a slow indirect-DMA gather of the full table.
All 8 cores run the identical program on their own batch shard (SPMD, no
collectives); host concatenates the 8 (64,1) outputs.
"""

import numpy as np
import ml_dtypes

import concourse.bacc as bacc
import concourse.bass as bass
import concourse.tile as tile
import concourse.mybir as mybir
from concourse.bass import IndirectOffsetOnAxis
from concourse.bass_utils import run_bass_kernel_spmd
from concourse import library_config

F32 = mybir.dt.float32
BF16 = mybir.dt.bfloat16
I16 = mybir.dt.int16
I32 = mybir.dt.int32
AF = mybir.ActivationFunctionType

# Problem shapes (hardcoded per spec)
B, N, C, E = 512, 128, 4, 300
V = 50000
H1, H2 = 512, 256
NF, NK = 8, 11
NCORES = 8
BC = B // NCORES          # 64 batches per core
EP = 384                  # padded embedding stride inside an x-row (768B)
CE = C * EP               # 1536 padded x-row length
KT = CE // 128            # 12 K-tiles for MLP1
CT = 32768                # compact table rows (int16-indexable)
GROUPS = (BC * N) // 512  # 16 groups of 512 (b,n) pairs
SUBT = 4                  # 128-bn subtiles per group
EB = 128                  # event-path width (64 real b + 64 junk)

MUS = [1.0, 0.9, 0.7, 0.5, 0.3, 0.1, -0.1, -0.3, -0.5, -0.7, -0.9]
SIGMAS = [1e-3] + [0.1] * 10

_PROGRAM_CACHE = {}


def _build_program(fast: bool):
    if fast in _PROGRAM_CACHE:
        return _PROGRAM_CACHE[fast]

    nc = bacc.Bacc("TRN2", target_bir_lowering=False, debug=False, num_swdge_queues=4)

    # ---- DRAM I/O ----
    if fast:
        ctab = nc.dram_tensor("ctab", (CT, EP), BF16, kind="ExternalInput")
        cidx = nc.dram_tensor("cidx", (128, GROUPS * 128), I16, kind="ExternalInput")
        eidx = nc.dram_tensor("eidx", (128, 32), I16, kind="ExternalInput")
    else:
        ctab = nc.dram_tensor("table", (V + 1, E), F32, kind="ExternalInput")
        cidx = nc.dram_tensor("ctxidx", (128, BC * C), I32, kind="ExternalInput")
        eidx = nc.dram_tensor("evidx", (BC, C), I32, kind="ExternalInput")
    w1t = nc.dram_tensor("w1t", (CE, H1), BF16, kind="ExternalInput")
    w2t = nc.dram_tensor("w2t", (H1, H2), BF16, kind="ExternalInput")
    wvt = nc.dram_tensor("wvt", (CE, 9), BF16, kind="ExternalInput")
    b1d = nc.dram_tensor("b1d", (128, 4), F32, kind="ExternalInput")
    b2d = nc.dram_tensor("b2d", (128, 2), F32, kind="ExternalInput")
    bvd = nc.dram_tensor("bvd", (9, 1), F32, kind="ExternalInput")
    wct = nc.dram_tensor("wct", (128, 1), F32, kind="ExternalInput")
    wckp = nc.dram_tensor("wckp", (1, NK), F32, kind="ExternalInput")
    bcd = nc.dram_tensor("bcd", (1, 1), F32, kind="ExternalInput")
    ndsq = nc.dram_tensor("ndsq", (9, BC), F32, kind="ExternalInput")
    featT = nc.dram_tensor("featT", (NF, BC), F32, kind="ExternalInput")
    out_d = nc.dram_tensor("out", (BC, 1), F32, kind="ExternalOutput")

    with tile.TileContext(nc) as tc:
        with (
            tc.tile_pool(name="consts", bufs=1) as cpool,
            tc.tile_pool(name="xg", bufs=4) as xgpool,
            tc.tile_pool(name="xt", bufs=4) as xtpool,
            tc.tile_pool(name="s1", bufs=8) as s1pool,
            tc.tile_pool(name="s2", bufs=4) as s2pool,
            tc.tile_pool(name="csq", bufs=4) as csqpool,
            tc.tile_pool(name="small", bufs=2) as smpool,
            tc.tile_pool(name="pm1", bufs=2, space="PSUM") as pm1,
            tc.tile_pool(name="pm2", bufs=2, space="PSUM") as pm2,
            tc.tile_pool(name="ptn", bufs=1, space="PSUM") as ptn,
            tc.tile_pool(name="pmisc", bufs=2, space="PSUM") as pmisc,
        ):
            # ---- load constants ----
            if fast:
                nc.gpsimd.load_library(library_config.mlp)
            w1t_s = cpool.tile([128, KT * H1], BF16)
            nc.sync.dma_start(
                w1t_s[:].rearrange("p (t m) -> p t m", t=KT),
                w1t.ap().rearrange("(t p) m -> p t m", p=128),
            )
            w2t_s = cpool.tile([128, 4 * H2], BF16)
            nc.sync.dma_start(
                w2t_s[:].rearrange("p (t m) -> p t m", t=4),
                w2t.ap().rearrange("(t p) m -> p t m", p=128),
            )
            wvt_s = cpool.tile([128, KT * 9], BF16)
            nc.sync.dma_start(
                wvt_s[:].rearrange("p (t m) -> p t m", t=KT),
                wvt.ap().rearrange("(t p) m -> p t m", p=128),
            )
            b1_s = cpool.tile([128, 4], F32)
            nc.sync.dma_start(b1_s[:], b1d.ap())
            b2_s = cpool.tile([128, 2], F32)
            nc.sync.dma_start(b2_s[:], b2d.ap())
            bv_s = cpool.tile([9, 1], F32)
            nc.sync.dma_start(bv_s[:], bvd.ap())
            wct_s = cpool.tile([128, 1], F32)
            nc.sync.dma_start(wct_s[:], wct.ap())
            wckp_s = cpool.tile([1, NK], F32)
            nc.sync.dma_start(wckp_s[:], wckp.ap())
            bc_s = cpool.tile([1, 1], F32)
            nc.sync.dma_start(bc_s[:], bcd.ap())
            if fast:
                cidx_s = cpool.tile([128, GROUPS * 128], I16)
                nc.sync.dma_start(cidx_s[:], cidx.ap())
                eidx_s = cpool.tile([128, 32], I16)
                nc.sync.dma_start(eidx_s[:], eidx.ap())
            else:
                cidx_s = cpool.tile([128, BC * C], I32)
                nc.sync.dma_start(cidx_s[:], cidx.ap())
                eidx_s = cpool.tile([BC, C], I32)
                nc.sync.dma_start(eidx_s[:], eidx.ap())
            ndsq_s = cpool.tile([9, BC], F32)
            nc.sync.dma_start(ndsq_s[:], ndsq.ap())
            feat_s = cpool.tile([128, BC], F32)
            nc.vector.memset(feat_s[:], 0.0)
            nc.sync.dma_start(feat_s[64 : 64 + NF, :], featT.ap())
            ones_s = cpool.tile([128, 1], BF16)
            nc.vector.memset(ones_s[:], 1.0)
            onesrow_s = cpool.tile([1, 128], F32)
            nc.vector.memset(onesrow_s[:], 1.0)
            onesf_s = cpool.tile([128, 1], F32)
            nc.vector.memset(onesf_s[:], 1.0)
            eps_s = cpool.tile([128, 1], F32)
            nc.vector.memset(eps_s[:], 1e-20)
            mub_s = cpool.tile([128, NK], F32)
            for k in range(NK):
                nc.vector.memset(mub_s[:, k : k + 1], -MUS[k])

            # ---- event path (EB=128 lanes, only 0..63 meaningful) ----
            xeT = cpool.tile([128, KT * EB], BF16)
            if fast:
                # transpose-mode gather lands K-major directly:
                # xeT[p, jj, c*128+b] = emb_{b,c}[jj*128+p]
                nc.gpsimd.dma_gather(
                    out_ap=xeT[:].rearrange("p (j i) -> p j i", j=3),
                    in_ap=ctab.ap(),
                    idxs_ap=eidx_s[:],
                    num_idxs=512,
                    num_idxs_reg=512,
                    elem_size=EP,
                    transpose=True,
                )
            else:
                xe = cpool.tile([EB, CE], BF16)
                nc.vector.memset(xe[:], 0.0)
                nc.gpsimd.indirect_dma_start(
                    out=xe[0:BC, :].rearrange("p (c e) -> p c e", c=C)[:, :, 0:E],
                    out_offset=None,
                    in_=ctab.ap(),
                    in_offset=IndirectOffsetOnAxis(ap=eidx_s[:], axis=0),
                )
                nc.sync.dma_start_transpose(
                    xeT[:].rearrange("p (j i) -> p j i", j=KT), xe[:]
                )

            def xeT_k(j):
                # K-tile j = 3*c + jj of the event activations
                if fast:
                    return xeT[:, 512 * (j % 3) + 128 * (j // 3) :][:, 0:128]
                return xeT[:, EB * j : EB * (j + 1)]

            s1e = cpool.tile([128, 4 * EB], BF16)
            for m in range(4):
                pe = pmisc.tile([128, EB], F32, tag="pmisc", name="pe")
                for j in range(KT):
                    nc.tensor.matmul(
                        pe[:],
                        w1t_s[:, H1 * j + 128 * m : H1 * j + 128 * m + 128],
                        xeT_k(j),
                        start=(j == 0),
                        stop=(j == KT - 1),
                    )
                nc.scalar.activation(
                    s1e[:, EB * m : EB * (m + 1)], pe[:], AF.Relu,
                    bias=b1_s[:, m : m + 1],
                )

            eh2 = [
                cpool.tile([128, EB], BF16, tag=f"eh2_{k}", name=f"eh2_{k}")
                for k in range(2)
            ]
            for m in range(2):
                pe2 = pmisc.tile([128, EB], F32, tag="pmisc", name="pe2")
                for j in range(4):
                    nc.tensor.matmul(
                        pe2[:],
                        w2t_s[:, H2 * j + 128 * m : H2 * j + 128 * m + 128],
                        s1e[:, EB * j : EB * (j + 1)],
                        start=(j == 0),
                        stop=(j == 3),
                    )
                nc.scalar.activation(
                    eh2[m][:], pe2[:], AF.Relu, bias=b2_s[:, m : m + 1]
                )

            # variances -> dist_emb rows 32..40 of feat_s
            pv = pmisc.tile([9, EB], F32, tag="pmisc", name="pv")
            for j in range(KT):
                nc.tensor.matmul(
                    pv[:],
                    wvt_s[:, 9 * j : 9 * (j + 1)],
                    xeT_k(j),
                    start=(j == 0),
                    stop=(j == KT - 1),
                )
            ez_s = smpool.tile([9, EB], F32)
            nc.scalar.activation(ez_s[:], pv[:], AF.Exp, bias=bv_s[:])
            ez1_s = smpool.tile([9, EB], F32)
            nc.vector.tensor_scalar_add(ez1_s[:], ez_s[:], 1.0)
            var_s = smpool.tile([9, EB], F32)
            nc.scalar.activation(var_s[:], ez1_s[:], AF.Ln)
            rv_s = smpool.tile([9, EB], F32)
            nc.vector.reciprocal(rv_s[:], var_s[:])
            q_s = smpool.tile([9, BC], F32)
            nc.vector.tensor_mul(q_s[:], ndsq_s[:], rv_s[:, 0:BC])
            nc.scalar.activation(feat_s[32:41, :], q_s[:], AF.Exp)

            # |e|^2 per b, broadcast to all 128 partitions via outer product
            esq = [
                smpool.tile([128, EB], BF16, tag=f"esq_{k}", name=f"esq_{k}")
                for k in range(2)
            ]
            for k in range(2):
                nc.vector.tensor_mul(esq[k][:], eh2[k][:], eh2[k][:])
            pne = pmisc.tile([1, EB], F32, tag="pmisc", name="pne")
            for k in range(2):
                nc.tensor.matmul(
                    pne[:], ones_s[:], esq[k][:], start=(k == 0), stop=(k == 1)
                )
            ne2_s = smpool.tile([1, BC], F32)
            nc.scalar.copy(ne2_s[:], pne[:, 0:BC])
            pne2bc = pmisc.tile([128, BC], F32, tag="pmisc", name="pne2bc")
            nc.tensor.matmul(
                pne2bc[:], onesrow_s[:], ne2_s[:], start=True, stop=True
            )
            ne2bc_s = cpool.tile([128, BC], F32)
            nc.scalar.copy(ne2bc_s[:], pne2bc[:])

            # persistent SBUF accumulators, n on partitions, b on free
            traw_s = cpool.tile([128, BC], F32)
            ncsq_s = cpool.tile([128, BC], F32)

            # ---- context groups ----
            for g in range(GROUPS):
                xt = xtpool.tile([128, KT * 512], BF16)
                if fast:
                    # per subtile s: xt[p, s, jj, c*128+pbn] (s-major blocks)
                    for s in range(SUBT):
                        nc.gpsimd.dma_gather(
                            out_ap=xt[:]
                            .rearrange("p (z j i) -> p z j i", z=SUBT, j=3)[
                                :, s, :, :
                            ],
                            in_ap=ctab.ap(),
                            idxs_ap=cidx_s[
                                :, 32 * (SUBT * g + s) : 32 * (SUBT * g + s + 1)
                            ],
                            num_idxs=512,
                            num_idxs_reg=512,
                            elem_size=EP,
                            transpose=True,
                        )
                else:
                    xg = xgpool.tile([128, SUBT * CE], BF16)
                    nc.vector.memset(
                        xg[:].rearrange("p (q e) -> p q e", e=EP)[:, :, E:EP],
                        0.0,
                    )
                    for s in range(SUBT):
                        nc.gpsimd.indirect_dma_start(
                            out=xg[:]
                            .rearrange("p (q c e) -> p q c e", q=SUBT, c=C)[
                                :, s, :, 0:E
                            ],
                            out_offset=None,
                            in_=ctab.ap(),
                            in_offset=IndirectOffsetOnAxis(
                                ap=cidx_s[
                                    :, (SUBT * g + s) * C : (SUBT * g + s + 1) * C
                                ],
                                axis=0,
                            ),
                        )
                    for s in range(SUBT):
                        nc.sync.dma_start_transpose(
                            xt[:].rearrange(
                                "p (j z i) -> p j z i", j=KT, z=SUBT
                            )[:, :, s, :],
                            xg[:, CE * s : CE * (s + 1)],
                        )

                def xt_k(j):
                    # K-tile j = 3*c + jj; cols ordered (s, pbn)
                    if fast:
                        off = 512 * (j % 3) + 128 * (j // 3)
                        return xt[:].rearrange(
                            "p (z x) -> p z x", z=SUBT
                        )[:, :, off : off + 128]
                    return xt[:, 512 * j : 512 * (j + 1)]

                s1 = [
                    s1pool.tile([128, 512], BF16, tag=f"s1_{m}", name=f"s1_{m}")
                    for m in range(4)
                ]
                for m in range(4):
                    p1 = pm1.tile([128, 512], F32)
                    for j in range(KT):
                        nc.tensor.matmul(
                            p1[:],
                            w1t_s[:, H1 * j + 128 * m : H1 * j + 128 * m + 128],
                            xt_k(j),
                            start=(j == 0),
                            stop=(j == KT - 1),
                        )
                    nc.scalar.activation(
                        s1[m][:], p1[:], AF.Relu, bias=b1_s[:, m : m + 1]
                    )

                s2 = [
                    s2pool.tile([128, 512], BF16, tag=f"s2_{m}", name=f"s2_{m}")
                    for m in range(2)
                ]
                for m in range(2):
                    p2 = pm2.tile([128, 512], F32)
                    for j in range(4):
                        nc.tensor.matmul(
                            p2[:],
                            w2t_s[:, H2 * j + 128 * m : H2 * j + 128 * m + 128],
                            s1[j][:],
                            start=(j == 0),
                            stop=(j == 3),
                        )
                    nc.scalar.activation(
                        s2[m][:], p2[:], AF.Relu, bias=b2_s[:, m : m + 1]
                    )

                csq = [
                    csqpool.tile([128, 512], BF16, tag=f"csq_{m}", name=f"csq_{m}")
                    for m in range(2)
                ]
                for m in range(2):
                    nc.vector.tensor_mul(csq[m][:], s2[m][:], s2[m][:])

                # raw dots and |c|^2, n on partitions, one column per b
                pT = ptn.tile([128, SUBT], F32, tag="pT", name="pT")
                pN = ptn.tile([128, SUBT], F32, tag="pN", name="pN")
                for s in range(SUBT):
                    b = SUBT * g + s
                    for k in range(2):
                        nc.tensor.matmul(
                            pT[:, s : s + 1],
                            s2[k][:, 128 * s : 128 * (s + 1)],
                            eh2[k][:, b : b + 1],
                            start=(k == 0),
                            stop=(k == 1),
                        )
                    for k in range(2):
                        nc.tensor.matmul(
                            pN[:, s : s + 1],
                            csq[k][:, 128 * s : 128 * (s + 1)],
                            ones_s[:],
                            start=(k == 0),
                            stop=(k == 1),
                        )
                nc.scalar.copy(traw_s[:, SUBT * g : SUBT * (g + 1)], pT[:])
                nc.scalar.copy(ncsq_s[:, SUBT * g : SUBT * (g + 1)], pN[:])

            # ---- kernel pooling (tiles are [n=128, b=64]) ----
            prodn = smpool.tile([128, BC], F32, tag="prodn")
            nc.vector.tensor_mul(prodn[:], ncsq_s[:], ne2bc_s[:])
            lnp = smpool.tile([128, BC], F32, tag="lnp")
            nc.scalar.activation(lnp[:], prodn[:], AF.Ln, bias=eps_s[:])
            nrmf = smpool.tile([128, BC], F32, tag="nrmf")
            nc.scalar.activation(nrmf[:], lnp[:], AF.Exp, scale=-0.5)
            trans = cpool.tile([128, BC], F32)
            nc.vector.tensor_mul(trans[:], traw_s[:], nrmf[:])

            kpp_s = cpool.tile([1, NK * BC], F32)
            for k in range(NK):
                sq = smpool.tile([128, BC], F32, tag="sq", name="sq")
                nc.scalar.activation(
                    sq[:], trans[:], AF.Square, bias=mub_s[:, k : k + 1]
                )
                arg = smpool.tile([128, BC], F32, tag="arg", name="arg")
                nc.vector.tensor_scalar(
                    arg[:], sq[:],
                    -1.0 / (2.0 * SIGMAS[k] ** 2), -87.0,
                    mybir.AluOpType.mult, mybir.AluOpType.max,
                )
                ek = smpool.tile([128, BC], F32, tag="ek", name="ek")
                nc.scalar.activation(ek[:], arg[:], AF.Exp)
                pp = pmisc.tile([1, BC], F32, tag="pmisc", name="pp")
                nc.tensor.matmul(pp[:], onesf_s[:], ek[:], start=True, stop=True)
                nc.scalar.copy(kpp_s[:, BC * k : BC * (k + 1)], pp[:])

            kpc_s = smpool.tile([1, NK * BC], F32, tag="kpc")
            nc.vector.tensor_scalar_max(kpc_s[:], kpp_s[:], 1e-10)
            kpl_s = smpool.tile([1, NK * BC], F32, tag="kpl")
            nc.scalar.activation(kpl_s[:], kpc_s[:], AF.Ln)

            # weighted sum over k: kps[b] = sum_k wckp[k] * kpl[k, b]
            kpw_s = smpool.tile([1, BC * NK], F32, tag="kpw")
            kpl_v = kpl_s[:].rearrange("p (k b) -> p b k", k=NK)
            wck_v = wckp_s[:][:, None, :].broadcast_to([1, BC, NK])
            kpw_v = kpw_s[:].rearrange("p (b k) -> p b k", b=BC)
            nc.vector.tensor_tensor(
                out=kpw_v, in0=kpl_v, in1=wck_v, op=mybir.AluOpType.mult
            )
            kps_s = smpool.tile([1, BC], F32, tag="kps")
            nc.vector.reduce_sum(
                out=kps_s[:], in_=kpw_v, axis=mybir.AxisListType.X
            )

            # ---- final score ----
            psc = pmisc.tile([1, BC], F32, tag="pmisc", name="psc")
            nc.tensor.matmul(psc[:], wct_s[:], feat_s[:], start=True, stop=True)
            tot_s = smpool.tile([1, BC], F32, tag="tot")
            nc.vector.tensor_add(tot_s[:], psc[:], kps_s[:])
            emx = smpool.tile([1, BC], F32, tag="emx")
            nc.scalar.activation(emx[:], tot_s[:], AF.Exp, bias=bc_s[:], scale=-1.0)
            emx1 = smpool.tile([1, BC], F32, tag="emx1")
            nc.vector.tensor_scalar_add(emx1[:], emx[:], 1.0)
            outs = smpool.tile([1, BC], F32, tag="outs")
            nc.vector.reciprocal(outs[:], emx1[:])
            nc.sync.dma_start(out_d.ap().rearrange("b one -> one b"), outs[:])

    nc.compile()

    # Spread SWDGE gathers across the 4 queues. The ucode locks each DMASW
    # semaphore lane to one queue, and Tile assigns lanes round-robin in
    # scheduled order, so derive queue from the assigned lane post-compile.
    import re as _re
    for blk in nc.m.functions[0].blocks:
        for inst in blk.instructions:
            if type(inst).__name__ == "InstDMAGatherAnt":
                for u in inst.sync_info.on_update:
                    m = _re.match(r"DMASW(\d+)_", u.ant_name or "")
                    if m:
                        inst.queue_num = int(m.group(1)) % 4
                        break

    _PROGRAM_CACHE[fast] = nc
    return nc


def _wrap16(flat_idx):
    """int16 index list -> (128, n/16) tile layout: unwrapped[i] =
    tile[i % 16, i // 16], replicated into all 8 16-partition stripes."""
    n = flat_idx.shape[0]
    t = np.zeros((16, n // 16), np.int16)
    t[np.arange(n) % 16, np.arange(n) // 16] = flat_idx
    return np.tile(t, (8, 1))


def _prep_core_inputs(inputs, core, fast):
    """Host-side shard + weight re-layouts for one core."""
    W1 = np.asarray(inputs["W1"], np.float32)
    W2 = np.asarray(inputs["W2"], np.float32)
    Wv = np.asarray(inputs["Wv"], np.float32)
    Wc = np.asarray(inputs["Wc"], np.float32)
    b1 = np.asarray(inputs["b1"], np.float32)
    b2 = np.asarray(inputs["b2"], np.float32)
    bv = np.asarray(inputs["bv"], np.float32)
    bc = np.asarray(inputs["bc"], np.float32)

    sl = slice(core * BC, (core + 1) * BC)
    ev = np.asarray(inputs["batch_event"][sl], np.int64)          # (BC, C)
    feats = np.asarray(inputs["batch_features"][sl], np.float32)  # (BC, NF)
    dists = np.asarray(inputs["batch_distances"][sl], np.float32) # (BC, 9)
    ctx = np.asarray(inputs["batch_context"][sl], np.int64)       # (BC, N, C)

    bf = ml_dtypes.bfloat16
    # W1.T with K padded 300->EP per component, zeros in the pad rows
    w1t = np.zeros((CE, H1), np.float32)
    for c in range(C):
        w1t[EP * c : EP * c + E, :] = W1[:, E * c : E * (c + 1)].T
    wvt = np.zeros((CE, 9), np.float32)
    wvt[EP * 1 : EP * 1 + E, :] = Wv.T  # predicates = component 1

    wc_full = np.zeros((128,), np.float32)
    wc_full[32 : 32 + 9] = Wc[0, 0:9]          # dist_emb block
    wc_full[64 : 64 + NF] = Wc[0, 9 : 9 + NF]  # batch_features block
    wckp = (Wc[0, NF + 9 :] * 0.01).astype(np.float32)  # kp block, 0.01 folded

    m = {
        "w1t": w1t.astype(bf),
        "w2t": np.ascontiguousarray(W2.T).astype(bf),
        "wvt": wvt.astype(bf),
        "b1d": np.ascontiguousarray(b1.reshape(4, 128).T),
        "b2d": np.ascontiguousarray(b2.reshape(2, 128).T),
        "bvd": bv.reshape(9, 1),
        "wct": wc_full.reshape(-1, 1),
        "wckp": wckp.reshape(1, NK),
        "bcd": -bc.reshape(1, 1),
        "ndsq": np.ascontiguousarray(-(dists * dists).T),
        "featT": np.ascontiguousarray(feats.T),
    }

    if fast:
        table = np.asarray(inputs["event_table"])
        allidx = np.concatenate([ctx.reshape(-1), ev.reshape(-1)])
        uniq, inv = np.unique(allidx, return_inverse=True)
        assert len(uniq) <= CT
        ctab = np.zeros((CT, EP), bf)
        ctab[: len(uniq), :E] = np.asarray(table[uniq], np.float32)
        rctx = inv[: ctx.size].astype(np.int16).reshape(BC, N, C)
        rev = inv[ctx.size :].astype(np.int16).reshape(BC, C)

        # context: per (g, s) gather of 512 idx with i = c*128 + p
        ci = rctx.reshape(GROUPS, SUBT, N, C).transpose(0, 1, 3, 2)  # g,s,c,p
        cidx = np.concatenate(
            [
                _wrap16(ci[g, s].reshape(-1))
                for g in range(GROUPS)
                for s in range(SUBT)
            ],
            axis=1,
        )
        # event: i = c*128 + b; b >= BC -> row 0 junk
        ei = np.zeros((C, 128), np.int16)
        ei[:, :BC] = rev.T
        m["ctab"] = ctab
        m["cidx"] = np.ascontiguousarray(cidx)
        m["eidx"] = np.ascontiguousarray(_wrap16(ei.reshape(-1)))
    else:
        m["table"] = np.ascontiguousarray(
            np.asarray(inputs["event_table"], np.float32)
        )
        m["ctxidx"] = np.ascontiguousarray(
            ctx.astype(np.int32).transpose(1, 0, 2).reshape(128, BC * C)
        )
        m["evidx"] = ev.astype(np.int32)
    return m


def kernel(**inputs) -> np.ndarray:
    # fast path requires every shard's unique row count to fit int16
    fast = True
    ctx = np.asarray(inputs["batch_context"], np.int64)
    ev = np.asarray(inputs["batch_event"], np.int64)
    for core in range(NCORES):
        sl = slice(core * BC, (core + 1) * BC)
        nuniq = len(np.unique(np.concatenate(
            [ctx[sl].reshape(-1), ev[sl].reshape(-1)])))
        if nuniq > CT:
            fast = False
            break
    nc = _build_program(fast)
    in_maps = [_prep_core_inputs(inputs, core, fast) for core in range(NCORES)]
    res = run_bass_kernel_spmd(nc, in_maps, core_ids=list(range(NCORES)))
    return np.concatenate([r["out"] for r in res.results], axis=0)


if __name__ == "__main__":
    nc = _build_program(True)
    print("program built ok")
